# revision 18
# baseline (speedup 1.0000x reference)
"""Trainium2 Bass kernel for nn_Classification_4922032521468.

Problem: acts = embeds[activity_index]  (A=512 rows, d=512)
         pairs = concat(acts[ii], acts[jj])  for all i<j (P=130816 pairs)
         out = log_softmax(pairs @ W.T + b)  -> [P, 4]

Key algebra: logits[p, c] = L[i, c] + R'[j, c]  with
  L  = acts @ Wl.T          (Wl = W[:, :512])
  R' = acts @ Wr.T + b      (Wr = W[:, 512:])
so log_softmax needs only lse[i, j] = ln(sum_c e^{L[i,c]} e^{R'[j,c]})
(a K=4 PE matmul of U = e^L rows against V = e^{R'}) and
  out[i, j, c] = L[i, c] + R'[j, c] - lse[i, j].
No 130816x1024 pair tensor is ever built.

v3 design (vs the fp32 baseline):
- fp16 data path end to end (PE streams 4x faster than fp32, DVE 2x,
  half the DMA bytes); PSUM accumulation stays fp32.
- One manual ACT table load of the combined exp+ln function set, so
  there is no mid-kernel 1.28us Exp->Ln table swap.
- Per-j-block software pipeline: gather block -> PE transpose -> PSUM
  copy -> projection matmuls accumulate R'^T columns, so compute rides
  the gather instead of waiting for it.
- Output plane per core is [512 j, 4 c, 64 i] (class-major), which
  makes the L broadcast four tiny K=4 selector matmuls (no strided
  reorder DMAs) and keeps the final combine at two DVE ops per chunk.
- Stores alternate between the SP and ACT HWDGE queues.

Sharding: core k owns i-rows [64k, 64k+64). Same NEFF on all 8 cores
(SPMD); per-core behavior comes only from per-core DATA: activity_index
is rotated by -64k so each core's own i-rows are gathered rows 0..63.
Each core outputs [512 j, 4 c, 64 i] (j rotated); the host un-rotates
j, transposes, and gathers the triu pairs.
"""

import numpy as np

A = 512  # number of activity tokens
D = 512  # embedding dim
C = 4  # classes
NTOK = 4096  # embeds table rows
RB = 64  # i-rows per core
NCORES = 8

_program = None
_last_results = None  # BassKernelResults from the most recent run (profiling)


def _build_program():
    from contextlib import ExitStack

    import concourse.bacc as bacc
    import concourse.mybir as mybir
    import concourse.tile as tile
    from concourse.bass import IndirectOffsetOnAxis
    from concourse.tile_rust import add_dep_helper

    fp32 = mybir.dt.float32
    fp16 = mybir.dt.float16
    i32 = mybir.dt.int32
    AF = mybir.ActivationFunctionType
    SUB = mybir.AluOpType.subtract
    ADD = mybir.AluOpType.add

    nc = bacc.Bacc(
        "TRN2",
        target_bir_lowering=False,
        debug=False,
        enable_asserts=False,
        num_devices=1,
    )

    embeds_h = nc.dram_tensor("embeds", (NTOK, D), fp16, kind="ExternalInput")
    # idxs[p, s] = rotated activity_index[128 s + p], int32
    idx_h = nc.dram_tensor("idxs", (128, 4), i32, kind="ExternalInput")
    # wt[d, 4k+c] = Wr.T[128k+d, c]; wt[d, 16+4k+c] = Wl.T[128k+d, c];
    # wt[0:4, 33:37] = I4
    wt_h = nc.dram_tensor("wt", (128, 40), fp16, kind="ExternalInput")
    b4_h = nc.dram_tensor("b4", (C, 1), fp32, kind="ExternalInput")
    # out[j, 64c + i] (j rotated per core)
    out_h = nc.dram_tensor("out", (A, RB * C), fp16, kind="ExternalOutput")

    # onesel[k, 128c + p] = (k == c): selector rows for the L broadcast
    osel_np = np.zeros((C, 512), dtype=np.float16)
    for c in range(C):
        osel_np[c, 128 * c : 128 * (c + 1)] = 1.0
    osel_h = nc.inline_tensor(osel_np, name="onesel")
    ident_h = nc.inline_tensor(np.eye(128, dtype=np.float16), name="ident")

    embeds_ap = embeds_h.ap()
    out_ap = out_h.ap()

    with tile.TileContext(nc) as tc, ExitStack() as ctx:
        sb = ctx.enter_context(tc.tile_pool(name="sb", bufs=1))
        sbc = ctx.enter_context(tc.tile_pool(name="sbc", bufs=16))
        sbr = ctx.enter_context(tc.tile_pool(name="sbr", bufs=4))
        psV = ctx.enter_context(tc.tile_pool(name="psV", bufs=1, space="PSUM"))
        psU = ctx.enter_context(tc.tile_pool(name="psU", bufs=1, space="PSUM"))
        psB = ctx.enter_context(tc.tile_pool(name="psB", bufs=1, space="PSUM"))
        psT = ctx.enter_context(tc.tile_pool(name="psT", bufs=2, space="PSUM"))
        psJ = ctx.enter_context(tc.tile_pool(name="psJ", bufs=1, space="PSUM"))
        psS = ctx.enter_context(tc.tile_pool(name="psS", bufs=2, space="PSUM"))

        # ---- input DMAs (idx first: it heads the gather dependency chain) --
        idxs = sb.tile([128, 4], i32, tag="idxs")
        nc.sync.dma_start(out=idxs[:], in_=idx_h.ap()[:])
        wtsb = sb.tile([128, 40], fp16, tag="wt")
        nc.sync.dma_start(out=wtsb[:], in_=wt_h.ap()[:])
        osel = sb.tile([C, 512], fp16, tag="osel")
        nc.sync.dma_start(out=osel[:], in_=osel_h.ap()[:])
        b4t = sb.tile([C, 1], fp32, tag="b4")
        nc.sync.dma_start(out=b4t[:], in_=b4_h.ap()[:])
        ident = sb.tile([128, 128], fp16, tag="ident")
        nc.scalar.dma_start(out=ident[:], in_=ident_h.ap()[:])

        # one combined exp+ln ACT table load, issued up front
        ldtab = nc.scalar.add_instruction(
            mybir.InstLoadActFuncSet(
                act_func_set_id=6,  # natural_log_exp_and_others
                name=f"I-{nc.next_id()}",
                engine=mybir.EngineType.Activation,
            )
        )

        b4 = b4t[:]
        i4 = wtsb[0:C, 33:37]

        PRv = psV.tile([C, A], fp32, tag="PRv")
        PRu = psU.tile([C, RB], fp32, tag="PRu")
        vt = sb.tile([C, A], fp16, tag="vt")  # e^{R'+b}, classes on K
        rt = sb.tile([C, A], fp16, tag="rt")  # R'+b (pre-ln logits, R side)

        # ---- PE warmup: >4us of dummy matmuls while the gather is in
        # flight, so the HAM clock gate opens (1.2 -> 2.4 GHz) before the
        # real transposes/matmuls arrive. The source tile is memset (no DMA
        # dependency) so the stream starts the moment the body is entered.
        wsrc = sb.tile([128, 128], fp16, tag="wsrc")
        nc.vector.memset(wsrc[:], 1.0)
        warm = psS.tile([128, RB], fp32, tag="se", name="warm")
        for w in range(80):
            nc.tensor.matmul(
                out=warm[:],
                lhsT=wsrc[:],
                rhs=wsrc[:, 0:RB],
                start=True,
                stop=True,
            )

        # ---- gather acts per 128-row j-block, then transpose, project, exp --
        blocks = []
        for jb in range(4):
            acts_b = sbc.tile([128, D], fp16, tag=f"acts{jb}", name=f"acts{jb}")
            nc.gpsimd.indirect_dma_start(
                out=acts_b[:],
                out_offset=None,
                in_=embeds_ap[:],
                in_offset=IndirectOffsetOnAxis(ap=idxs[:, jb : jb + 1], axis=0),
            )
            blocks.append(acts_b)

        at0 = None
        for jb in range(4):
            acts_b = blocks[jb][:]
            pt = psT.tile([128, 4, 128], fp16, tag="pt", name="pt")
            for k in range(4):
                nc.tensor.transpose(
                    out=pt[:, k, :],
                    in_=acts_b[:, 128 * k : 128 * (k + 1)],
                    identity=ident[:],
                )
            at = sbc.tile([128, 4, 128], fp16, tag="at", name="at")
            nc.vector.tensor_copy(out=at[:], in_=pt[:])
            if jb == 0:
                at0 = at
            for k in range(4):
                nc.tensor.matmul(
                    out=PRv[:, 128 * jb : 128 * (jb + 1)],
                    lhsT=wtsb[:, 4 * k : 4 * k + 4],
                    rhs=at[:, k, :],
                    start=(k == 0),
                    stop=(k == 3),
                )
            e = nc.scalar.activation(
                out=vt[:, 128 * jb : 128 * (jb + 1)],
                in_=PRv[:, 128 * jb : 128 * (jb + 1)],
                func=AF.Exp,
                bias=b4,
            )
            if jb == 0:
                add_dep_helper(e.ins, ldtab.ins, sync=False, reason="act-table")
            nc.scalar.activation(
                out=rt[:, 128 * jb : 128 * (jb + 1)],
                in_=PRv[:, 128 * jb : 128 * (jb + 1)],
                func=AF.Identity,
                bias=b4,
            )

        # ---- own-rows L projection (block 0 cols 0:64) ----
        for k in range(4):
            nc.tensor.matmul(
                out=PRu[:],
                lhsT=wtsb[:, 16 + 4 * k : 16 + 4 * k + 4],
                rhs=at0[:, k, 0:RB],
                start=(k == 0),
                stop=(k == 3),
            )
        ut = sb.tile([C, RB], fp16, tag="ut")  # e^{L}, own rows
        eu = nc.scalar.activation(out=ut[:], in_=PRu[:], func=AF.Exp)
        add_dep_helper(eu.ins, ldtab.ins, sync=False, reason="act-table")
        lt4 = sb.tile([C, RB], fp16, tag="lt4")  # L^T own rows
        nc.vector.tensor_copy(out=lt4[:], in_=PRu[:])

        # ---- L broadcast plane lbb[p, 64c+i] = L[i, c] (selector matmuls) --
        lbb = psB.tile([128, RB * C], fp32, tag="lbb")
        for c in range(C):
            nc.tensor.matmul(
                out=lbb[:, RB * c : RB * (c + 1)],
                lhsT=osel[:, 128 * c : 128 * (c + 1)],
                rhs=lt4[:],
                start=True,
                stop=True,
            )
        lbb3 = lbb[:].rearrange("p (c i) -> p c i", c=C)

        # ---- rj: R'+b row-major per j-chunk ----
        rjsb = sb.tile([128, 16], fp16, tag="rjsb")
        for jc in range(4):
            pj = psJ.tile([128, C], fp16, tag="pj", name="pj")
            nc.tensor.transpose(
                out=pj[:], in_=rt[:, 128 * jc : 128 * (jc + 1)], identity=i4
            )
            nc.vector.tensor_copy(out=rjsb[:, 4 * jc : 4 * jc + 4], in_=pj[:])

        # ---- per j-chunk: lse, combine, store ----
        for jc in range(4):
            se = psS.tile([128, RB], fp32, tag="se", name="se")
            nc.tensor.matmul(
                out=se[:],
                lhsT=vt[:, 128 * jc : 128 * (jc + 1)],
                rhs=ut[:],
                start=True,
                stop=True,
            )
            lnse = sbr.tile([128, RB], fp32, tag="lnse", name="lnse")
            ln_i = nc.scalar.activation(out=lnse[:], in_=se[:], func=AF.Ln)
            if jc == 0:
                add_dep_helper(ln_i.ins, ldtab.ins, sync=False, reason="act-table")

            t1 = sbr.tile([128, RB * C], fp16, tag="t1", name="t1")
            nc.vector.tensor_tensor(
                out=t1[:].rearrange("p (c i) -> p c i", c=C),
                in0=lbb3,
                in1=lnse[:].unsqueeze(1).to_broadcast([128, C, RB]),
                op=SUB,
            )
            oj = sbr.tile([128, RB * C], fp16, tag="oj", name="oj")
            nc.vector.tensor_tensor(
                out=oj[:].rearrange("p (c i) -> p c i", c=C),
                in0=t1[:].rearrange("p (c i) -> p c i", c=C),
                in1=rjsb[:, 4 * jc : 4 * jc + 4]
                .unsqueeze(2)
                .to_broadcast([128, C, RB]),
                op=ADD,
            )
            nc.sync.dma_start(out=out_ap[128 * jc : 128 * (jc + 1), :], in_=oj[:])

    nc.compile()
    return nc


def _get_program():
    global _program
    if _program is None:
        _program = _build_program()
    return _program


def _prep_core_inputs(embeds16, idx64, wt_np, b4v, k):
    # idxs[p, s] = rot[128 s + p]: column s feeds j-block s of the gather
    rot = np.roll(idx64, -RB * k)
    idxs = np.ascontiguousarray(rot.reshape(4, 128).T.astype(np.int32))
    return {"embeds": embeds16, "idxs": idxs, "wt": wt_np, "b4": b4v}


def kernel(embeds, activity_index, W, b):
    from concourse.bass_utils import run_bass_kernel_spmd

    embeds16 = np.ascontiguousarray(
        np.asarray(embeds, dtype=np.float32).astype(np.float16)
    )
    W = np.asarray(W, dtype=np.float32)
    b_in = np.asarray(b, dtype=np.float32).reshape(C)
    idx64 = np.asarray(activity_index).astype(np.int64)

    wt_np = np.zeros((128, 40), dtype=np.float16)
    for k in range(4):
        wt_np[:, 4 * k : 4 * k + 4] = W[:, D + 128 * k : D + 128 * (k + 1)].T
        wt_np[:, 16 + 4 * k : 16 + 4 * k + 4] = W[:, 128 * k : 128 * (k + 1)].T
    wt_np[0:C, 33:37] = np.eye(C, dtype=np.float16)
    wt_np = np.ascontiguousarray(wt_np)

    nc = _get_program()
    b4v = np.ascontiguousarray(b_in.reshape(C, 1))
    in_maps = [_prep_core_inputs(embeds16, idx64, wt_np, b4v, k) for k in range(NCORES)]

    results = run_bass_kernel_spmd(nc, in_maps, core_ids=list(range(NCORES)))
    global _last_results
    _last_results = results

    out_sq = np.empty((A, A, C), dtype=np.float32)
    for k in range(NCORES):
        # blk[jrot, c, i] -> un-rotate j, reorder to [i, j, c]
        blk = results.results[k]["out"].reshape(A, C, RB).astype(np.float32)
        out_sq[RB * k : RB * (k + 1)] = np.roll(blk, RB * k, axis=0).transpose(2, 0, 1)

    ii, jj = np.triu_indices(A, k=1)
    return np.ascontiguousarray(out_sq[ii, jj])


# revision 19
# speedup vs baseline: 1.0001x; 1.0001x over previous
"""Trainium2 Bass kernel for nn_Classification_4922032521468.

Problem: acts = embeds[activity_index]  (A=512 rows, d=512)
         pairs = concat(acts[ii], acts[jj])  for all i<j (P=130816 pairs)
         out = log_softmax(pairs @ W.T + b)  -> [P, 4]

Key algebra: logits[p, c] = L[i, c] + R'[j, c]  with
  L  = acts @ Wl.T          (Wl = W[:, :512])
  R' = acts @ Wr.T + b      (Wr = W[:, 512:])
so log_softmax needs only lse[i, j] = ln(sum_c e^{L[i,c]} e^{R'[j,c]})
(a K=4 PE matmul of U = e^L rows against V = e^{R'}) and
  out[i, j, c] = L[i, c] + R'[j, c] - lse[i, j].
No 130816x1024 pair tensor is ever built.

v3 design (vs the fp32 baseline):
- fp16 data path end to end (PE streams 4x faster than fp32, DVE 2x,
  half the DMA bytes); PSUM accumulation stays fp32.
- One manual ACT table load of the combined exp+ln function set, so
  there is no mid-kernel 1.28us Exp->Ln table swap.
- Per-j-block software pipeline: gather block -> PE transpose -> PSUM
  copy -> projection matmuls accumulate R'^T columns, so compute rides
  the gather instead of waiting for it.
- Output plane per core is [512 j, 4 c, 64 i] (class-major), which
  makes the L broadcast four tiny K=4 selector matmuls (no strided
  reorder DMAs) and keeps the final combine at two DVE ops per chunk.
- Stores alternate between the SP and ACT HWDGE queues.

Sharding: core k owns i-rows [64k, 64k+64). Same NEFF on all 8 cores
(SPMD); per-core behavior comes only from per-core DATA: activity_index
is rotated by -64k so each core's own i-rows are gathered rows 0..63.
Each core outputs [512 j, 4 c, 64 i] (j rotated); the host un-rotates
j, transposes, and gathers the triu pairs.
"""

import numpy as np

A = 512  # number of activity tokens
D = 512  # embedding dim
C = 4  # classes
NTOK = 4096  # embeds table rows
RB = 64  # i-rows per core
NCORES = 8

_program = None
_last_results = None  # BassKernelResults from the most recent run (profiling)


def _build_program():
    from contextlib import ExitStack

    import concourse.bacc as bacc
    import concourse.mybir as mybir
    import concourse.tile as tile
    from concourse.bass import IndirectOffsetOnAxis
    from concourse.tile_rust import add_dep_helper

    fp32 = mybir.dt.float32
    fp16 = mybir.dt.float16
    i32 = mybir.dt.int32
    AF = mybir.ActivationFunctionType
    SUB = mybir.AluOpType.subtract
    ADD = mybir.AluOpType.add

    nc = bacc.Bacc(
        "TRN2",
        target_bir_lowering=False,
        debug=False,
        enable_asserts=False,
        num_devices=1,
    )

    embeds_h = nc.dram_tensor("embeds", (NTOK, D), fp16, kind="ExternalInput")
    # idxs[p, s] = rotated activity_index[128 s + p], int32
    idx_h = nc.dram_tensor("idxs", (128, 4), i32, kind="ExternalInput")
    # wt[d, 4k+c] = Wr.T[128k+d, c]; wt[d, 16+4k+c] = Wl.T[128k+d, c];
    # wt[0:4, 33:37] = I4
    wt_h = nc.dram_tensor("wt", (128, 40), fp16, kind="ExternalInput")
    b4_h = nc.dram_tensor("b4", (C, 1), fp32, kind="ExternalInput")
    # out[j, 64c + i] (j rotated per core)
    out_h = nc.dram_tensor("out", (A, RB * C), fp16, kind="ExternalOutput")

    # onesel[k, 128c + p] = (k == c): selector rows for the L broadcast
    osel_np = np.zeros((C, 512), dtype=np.float16)
    for c in range(C):
        osel_np[c, 128 * c : 128 * (c + 1)] = 1.0
    osel_h = nc.inline_tensor(osel_np, name="onesel")
    ident_h = nc.inline_tensor(np.eye(128, dtype=np.float16), name="ident")

    embeds_ap = embeds_h.ap()
    out_ap = out_h.ap()

    with tile.TileContext(nc) as tc, ExitStack() as ctx:
        sb = ctx.enter_context(tc.tile_pool(name="sb", bufs=1))
        sbc = ctx.enter_context(tc.tile_pool(name="sbc", bufs=16))
        sbr = ctx.enter_context(tc.tile_pool(name="sbr", bufs=4))
        psV = ctx.enter_context(tc.tile_pool(name="psV", bufs=1, space="PSUM"))
        psU = ctx.enter_context(tc.tile_pool(name="psU", bufs=1, space="PSUM"))
        psB = ctx.enter_context(tc.tile_pool(name="psB", bufs=1, space="PSUM"))
        psT = ctx.enter_context(tc.tile_pool(name="psT", bufs=2, space="PSUM"))
        psJ = ctx.enter_context(tc.tile_pool(name="psJ", bufs=1, space="PSUM"))
        psS = ctx.enter_context(tc.tile_pool(name="psS", bufs=2, space="PSUM"))

        # ---- input DMAs (idx first: it heads the gather dependency chain) --
        idxs = sb.tile([128, 4], i32, tag="idxs")
        nc.sync.dma_start(out=idxs[:], in_=idx_h.ap()[:])
        wtsb = sb.tile([128, 40], fp16, tag="wt")
        nc.sync.dma_start(out=wtsb[:], in_=wt_h.ap()[:])
        osel = sb.tile([C, 512], fp16, tag="osel")
        nc.sync.dma_start(out=osel[:], in_=osel_h.ap()[:])
        b4t = sb.tile([C, 1], fp32, tag="b4")
        nc.sync.dma_start(out=b4t[:], in_=b4_h.ap()[:])
        ident = sb.tile([128, 128], fp16, tag="ident")
        nc.scalar.dma_start(out=ident[:], in_=ident_h.ap()[:])

        # one combined exp+ln ACT table load, issued up front
        ldtab = nc.scalar.add_instruction(
            mybir.InstLoadActFuncSet(
                act_func_set_id=6,  # natural_log_exp_and_others
                name=f"I-{nc.next_id()}",
                engine=mybir.EngineType.Activation,
            )
        )

        b4 = b4t[:]
        i4 = wtsb[0:C, 33:37]

        PRv = psV.tile([C, A], fp32, tag="PRv")
        PRu = psU.tile([C, RB], fp32, tag="PRu")
        vt = sb.tile([C, A], fp16, tag="vt")  # e^{R'+b}, classes on K
        rt = sb.tile([C, A], fp16, tag="rt")  # R'+b (pre-ln logits, R side)

        # ---- PE warmup: >4us of dummy matmuls while the gather is in
        # flight, so the HAM clock gate opens (1.2 -> 2.4 GHz) before the
        # real transposes/matmuls arrive. The source tile is memset (no DMA
        # dependency) so the stream starts the moment the body is entered.
        wsrc = sb.tile([128, 128], fp16, tag="wsrc")
        nc.vector.memset(wsrc[:], 1.0)
        warm = psS.tile([128, RB], fp32, tag="se", name="warm")
        for w in range(80):
            nc.tensor.matmul(
                out=warm[:],
                lhsT=wsrc[:],
                rhs=wsrc[:, 0:RB],
                start=True,
                stop=True,
            )

        # ---- gather acts per 128-row j-block, then transpose, project, exp --
        blocks = []
        for jb in range(4):
            acts_b = sbc.tile([128, D], fp16, tag=f"acts{jb}", name=f"acts{jb}")
            nc.gpsimd.indirect_dma_start(
                out=acts_b[:],
                out_offset=None,
                in_=embeds_ap[:],
                in_offset=IndirectOffsetOnAxis(ap=idxs[:, jb : jb + 1], axis=0),
            )
            blocks.append(acts_b)

        at0 = None
        for jb in range(4):
            acts_b = blocks[jb][:]
            pt = psT.tile([128, 4, 128], fp16, tag="pt", name="pt")
            for k in range(4):
                nc.tensor.transpose(
                    out=pt[:, k, :],
                    in_=acts_b[:, 128 * k : 128 * (k + 1)],
                    identity=ident[:],
                )
            at = sbc.tile([128, 4, 128], fp16, tag="at", name="at")
            nc.vector.tensor_copy(out=at[:], in_=pt[:])
            if jb == 0:
                at0 = at
            for k in range(4):
                nc.tensor.matmul(
                    out=PRv[:, 128 * jb : 128 * (jb + 1)],
                    lhsT=wtsb[:, 4 * k : 4 * k + 4],
                    rhs=at[:, k, :],
                    start=(k == 0),
                    stop=(k == 3),
                )
            e = nc.scalar.activation(
                out=vt[:, 128 * jb : 128 * (jb + 1)],
                in_=PRv[:, 128 * jb : 128 * (jb + 1)],
                func=AF.Exp,
                bias=b4,
            )
            if jb == 0:
                add_dep_helper(e.ins, ldtab.ins, sync=False, reason="act-table")
            nc.scalar.activation(
                out=rt[:, 128 * jb : 128 * (jb + 1)],
                in_=PRv[:, 128 * jb : 128 * (jb + 1)],
                func=AF.Identity,
                bias=b4,
            )

        # ---- own-rows L projection (block 0 cols 0:64) ----
        for k in range(4):
            nc.tensor.matmul(
                out=PRu[:],
                lhsT=wtsb[:, 16 + 4 * k : 16 + 4 * k + 4],
                rhs=at0[:, k, 0:RB],
                start=(k == 0),
                stop=(k == 3),
            )
        ut = sb.tile([C, RB], fp16, tag="ut")  # e^{L}, own rows
        eu = nc.scalar.activation(out=ut[:], in_=PRu[:], func=AF.Exp)
        add_dep_helper(eu.ins, ldtab.ins, sync=False, reason="act-table")
        lt4 = sb.tile([C, RB], fp16, tag="lt4")  # L^T own rows
        nc.vector.tensor_copy(out=lt4[:], in_=PRu[:])

        # ---- L broadcast plane lbb[p, 64c+i] = L[i, c] (selector matmuls) --
        lbb = psB.tile([128, RB * C], fp32, tag="lbb")
        for c in range(C):
            nc.tensor.matmul(
                out=lbb[:, RB * c : RB * (c + 1)],
                lhsT=osel[:, 128 * c : 128 * (c + 1)],
                rhs=lt4[:],
                start=True,
                stop=True,
            )
        lbb3 = lbb[:].rearrange("p (c i) -> p c i", c=C)

        # ---- rj: R'+b row-major per j-chunk ----
        rjsb = sb.tile([128, 16], fp16, tag="rjsb")
        for jc in range(4):
            pj = psJ.tile([128, C], fp16, tag="pj", name="pj")
            nc.tensor.transpose(
                out=pj[:], in_=rt[:, 128 * jc : 128 * (jc + 1)], identity=i4
            )
            nc.vector.tensor_copy(out=rjsb[:, 4 * jc : 4 * jc + 4], in_=pj[:])

        # ---- per j-chunk: lse, combine, store ----
        for jc in range(4):
            se = psS.tile([128, RB], fp32, tag="se", name="se")
            nc.tensor.matmul(
                out=se[:],
                lhsT=vt[:, 128 * jc : 128 * (jc + 1)],
                rhs=ut[:],
                start=True,
                stop=True,
            )
            lnse = sbr.tile([128, RB], fp32, tag="lnse", name="lnse")
            ln_i = nc.scalar.activation(out=lnse[:], in_=se[:], func=AF.Ln)
            if jc == 0:
                add_dep_helper(ln_i.ins, ldtab.ins, sync=False, reason="act-table")

            # pre = L + R' is independent of lnse, so it runs off the
            # critical exp->lse->ln chain; only one DVE op follows the Ln.
            t1 = sbr.tile([128, RB * C], fp16, tag="t1", name="t1")
            nc.vector.tensor_tensor(
                out=t1[:].rearrange("p (c i) -> p c i", c=C),
                in0=lbb3,
                in1=rjsb[:, 4 * jc : 4 * jc + 4]
                .unsqueeze(2)
                .to_broadcast([128, C, RB]),
                op=ADD,
            )
            oj = sbr.tile([128, RB * C], fp16, tag="oj", name="oj")
            nc.vector.tensor_tensor(
                out=oj[:].rearrange("p (c i) -> p c i", c=C),
                in0=t1[:].rearrange("p (c i) -> p c i", c=C),
                in1=lnse[:].unsqueeze(1).to_broadcast([128, C, RB]),
                op=SUB,
            )
            nc.sync.dma_start(out=out_ap[128 * jc : 128 * (jc + 1), :], in_=oj[:])

    nc.compile()
    return nc


def _get_program():
    global _program
    if _program is None:
        _program = _build_program()
    return _program


def _prep_core_inputs(embeds16, idx64, wt_np, b4v, k):
    # idxs[p, s] = rot[128 s + p]: column s feeds j-block s of the gather
    rot = np.roll(idx64, -RB * k)
    idxs = np.ascontiguousarray(rot.reshape(4, 128).T.astype(np.int32))
    return {"embeds": embeds16, "idxs": idxs, "wt": wt_np, "b4": b4v}


def kernel(embeds, activity_index, W, b):
    from concourse.bass_utils import run_bass_kernel_spmd

    embeds16 = np.ascontiguousarray(
        np.asarray(embeds, dtype=np.float32).astype(np.float16)
    )
    W = np.asarray(W, dtype=np.float32)
    b_in = np.asarray(b, dtype=np.float32).reshape(C)
    idx64 = np.asarray(activity_index).astype(np.int64)

    wt_np = np.zeros((128, 40), dtype=np.float16)
    for k in range(4):
        wt_np[:, 4 * k : 4 * k + 4] = W[:, D + 128 * k : D + 128 * (k + 1)].T
        wt_np[:, 16 + 4 * k : 16 + 4 * k + 4] = W[:, 128 * k : 128 * (k + 1)].T
    wt_np[0:C, 33:37] = np.eye(C, dtype=np.float16)
    wt_np = np.ascontiguousarray(wt_np)

    nc = _get_program()
    b4v = np.ascontiguousarray(b_in.reshape(C, 1))
    in_maps = [_prep_core_inputs(embeds16, idx64, wt_np, b4v, k) for k in range(NCORES)]

    results = run_bass_kernel_spmd(nc, in_maps, core_ids=list(range(NCORES)))
    global _last_results
    _last_results = results

    out_sq = np.empty((A, A, C), dtype=np.float32)
    for k in range(NCORES):
        # blk[jrot, c, i] -> un-rotate j, reorder to [i, j, c]
        blk = results.results[k]["out"].reshape(A, C, RB).astype(np.float32)
        out_sq[RB * k : RB * (k + 1)] = np.roll(blk, RB * k, axis=0).transpose(2, 0, 1)

    ii, jj = np.triu_indices(A, k=1)
    return np.ascontiguousarray(out_sq[ii, jj])
